# revision 7
# baseline (speedup 1.0000x reference)
"""
Trainium2 Bass kernel for batched cross-attention:
  context[b] = softmax(q[b] @ tokens[b].T / sqrt(d)) @ tokens[b]
with x_latent (tokens) [16, 4096, 768] f32, prompts_latent (q) [16, 64, 768] f32.

Sharding: data-parallel over the batch dim — 16 batches / 8 cores = 2 per core.

V2 design (all bf16; fp8 scores measured at 2.5-3.7e-2 rel err — over the gate):
  - tokens ship ONCE in natural layout tn [4096, 769] (ones column baked in for
    softmax denominators via mm2), partition-major-packed so each group load is
    128 contiguous 6.2KB descriptors.
  - mm1 needs T^T (d on partitions). For TR_GROUPS per batch the tt tiles are
    built on-chip: 24 PE transposes [128,128] per group (LS-rate ~110ns each)
    with PSUM->SBUF copies split DVE/ACT. The remaining groups ship tt from
    host (pre-transposed, packed) — balances DMA (~58us) vs PE (~58us).
  - mm1: S[64, 512] = sum_c qt[:,c,:].T @ tt_g[:,c,:] (6 matmuls, 512 moving).
  - softmax without max-subtraction; exp on ACT in [64,128] chunks; row sums
    come FREE from mm2's ones column (no DVE reduces).
  - P chunks PE-transposed to P^T [128, 64]; mm2: O[64, 769] += P^T.T @ tn
    tile (the 769th output column accumulates the softmax denominator).
  - final: O rows * reciprocal(denominator) on DVE, store f32.

Pipeline: 2-stage software pipeline across both batches (PE program order per
iteration: [Ttr(i) if unshipped] mm1(i), mm2(i-2), ptr(i-1)).
"""

import os
import sys

import numpy as np

for _p in ("/opt/trn_rl_repo", "/root/.axon_site/_ro/trn_rl_repo"):
    if os.path.isdir(_p) and _p not in sys.path:
        sys.path.append(_p)

import ml_dtypes
from contextlib import ExitStack

import concourse.bass as bass
import concourse.mybir as mybir
import concourse.tile as tile
from concourse import bacc
from concourse.bass_utils import run_bass_kernel_spmd
from concourse.masks import make_identity

BF16 = ml_dtypes.bfloat16

N_CORES = 8
B_TOTAL = 16
BPC = B_TOTAL // N_CORES  # batches per core
N = 4096  # tokens
D = 768   # latent dim
P = 64    # prompts
DC = D // 128   # d-chunks of 128 (contraction tiles for mm1)
NT = N // 128   # n-tiles of 128
G = N // 512    # groups of 512 columns for mm1/softmax
SCALE = float(D) ** -0.5

# groups whose tt tiles are built on-chip via PE transposes (rest ship tt)
TR_GROUPS = ()
SHIP_GROUPS = tuple(g for g in range(G) if g not in TR_GROUPS)
NSHIP = len(SHIP_GROUPS)
SHIP_IDX = {g: i for i, g in enumerate(SHIP_GROUPS)}

_cached_nc = None


def build_bass_program() -> bass.Bass:
    nc = bacc.Bacc("TRN2", target_bir_lowering=False, debug=False)
    # host-packed layouts: partition-major, fully contiguous per partition
    qt = nc.declare_dram_parameter("qt", [BPC, 128, DC * P], mybir.dt.bfloat16, isOutput=False)
    tt = nc.declare_dram_parameter("tt", [BPC, NSHIP, 128, DC * 512], mybir.dt.bfloat16, isOutput=False)
    tn = nc.declare_dram_parameter("tn", [BPC, G, 128, 4 * 769], mybir.dt.bfloat16, isOutput=False)
    out = nc.declare_dram_parameter("out", [BPC, P, D], mybir.dt.float32, isOutput=True)

    with tile.TileContext(nc) as tc, ExitStack() as ctx:
        singles = ctx.enter_context(tc.tile_pool(name="singles", bufs=1))
        qt_pool = ctx.enter_context(tc.tile_pool(name="qt", bufs=3))
        tt_pool = ctx.enter_context(tc.tile_pool(name="ttg", bufs=5))
        tn_pool = ctx.enter_context(tc.tile_pool(name="tnt", bufs=7))
        p_pool = ctx.enter_context(tc.tile_pool(name="pexp", bufs=4))
        pt_pool = ctx.enter_context(tc.tile_pool(name="ptT", bufs=12))
        sums_pool = ctx.enter_context(tc.tile_pool(name="sums", bufs=2))
        o_pool = ctx.enter_context(tc.tile_pool(name="osb", bufs=2))

        psum_s = ctx.enter_context(tc.tile_pool(name="psum_s", bufs=2, space="PSUM"))
        # shared by the P-chunk transposes ([128,64] out) and the on-chip
        # token-tile transposes ([128,128] out); PSUM is bank-granular so
        # every tile costs a 2KB bank regardless of size.
        psum_t = ctx.enter_context(tc.tile_pool(name="psum_t", bufs=4, space="PSUM"))
        psum_o = ctx.enter_context(tc.tile_pool(name="psum_o", bufs=1, space="PSUM"))

        ident = singles.tile([P, P], mybir.dt.bfloat16)
        make_identity(nc, ident)
        ident128 = singles.tile([128, 128], mybir.dt.bfloat16)
        make_identity(nc, ident128)

        qt_ts = [None] * BPC
        o_ab = [None] * BPC

        def transpose_stage(p_sb, b, g):
            # PE transposes of the 4 P chunks + DVE copies to SBUF.
            pts = []
            for j in range(4):
                pt_ps = psum_t.tile([128, 128], mybir.dt.bfloat16, tag="t")
                nc.tensor.transpose(pt_ps[:, 0:P], p_sb[:, j * 128:(j + 1) * 128], ident)
                pts.append(pt_ps)
            outs = []
            for j in range(4):
                pt_sb = pt_pool.tile([128, P], mybir.dt.bfloat16)
                nc.vector.tensor_copy(pt_sb, pts[j][:, 0:P])
                outs.append(pt_sb)
            return outs

        def mm2_stage(pt_sbs, tn_g, b, g):
            if o_ab[b] is None:
                o_a = psum_o.tile([P, 512], mybir.dt.float32, tag="o_a")
                o_b_ = psum_o.tile([P, 257], mybir.dt.float32, tag="o_b")
                o_ab[b] = (o_a, o_b_)
            o_a, o_b_ = o_ab[b]
            for j in range(4):
                nt = g * 4 + j
                nc.tensor.matmul(
                    o_a,
                    lhsT=pt_sbs[j],
                    rhs=tn_g[:, j, 0:512],
                    start=(nt == 0),
                    stop=(nt == NT - 1),
                )
                nc.tensor.matmul(
                    o_b_,
                    lhsT=pt_sbs[j],
                    rhs=tn_g[:, j, 512:769],
                    start=(nt == 0),
                    stop=(nt == NT - 1),
                )
            if g == G - 1:
                finish_batch(b)

        def finish_batch(b):
            # normalization + store; the denominator came for free out of
            # mm2's ones column (o_b col 256).
            o_a, o_b_ = o_ab[b]
            rec = sums_pool.tile([P, 1], mybir.dt.float32)
            nc.vector.reciprocal(rec, o_b_[:, 256:257])
            o_sb = o_pool.tile([P, D], mybir.dt.float32)
            nc.vector.tensor_scalar_mul(o_sb[:, 0:512], o_a, rec)
            nc.vector.tensor_scalar_mul(o_sb[:, 512:768], o_b_[:, 0:256], rec)
            nc.gpsimd.dma_start(out=out[b], in_=o_sb)

        # One continuous two-stage software pipeline across BOTH batches.
        tr_q = []   # (p_sb, tn_g, b, g) awaiting P-transpose stage (depth 2)
        mm2_q = []  # (pt_sbs, tn_g, b, g) awaiting mm2 stage
        for idx in range(BPC * G):
            b, g = divmod(idx, G)
            if g == 0:
                qt_ts[b] = qt_pool.tile([128, DC, P], mybir.dt.bfloat16, tag="qt_t", name="qt_t")
                nc.gpsimd.dma_start(
                    out=qt_ts[b], in_=qt[b].rearrange("p (c m) -> p c m", c=DC)
                )
            qt_t = qt_ts[b]

            tn_g = tn_pool.tile([128, 4, 769], mybir.dt.bfloat16)
            nc.scalar.dma_start(
                out=tn_g, in_=tn[b, g].rearrange("p (t d) -> p t d", t=4)
            )

            tt_g = tt_pool.tile([128, DC, 512], mybir.dt.bfloat16)
            if g in TR_GROUPS:
                # build tt tiles on-chip: PE transposes of tn sub-tiles,
                # copies interleaved DVE/ACT.
                k = 0
                for c in range(DC):
                    ps_l = []
                    for j in range(4):
                        ps = psum_t.tile([128, 128], mybir.dt.bfloat16, tag="t")
                        nc.tensor.transpose(
                            ps, tn_g[:, j, c * 128:(c + 1) * 128], ident128
                        )
                        ps_l.append(ps)
                    for j in range(4):
                        dst = tt_g[:, c, j * 128:(j + 1) * 128]
                        if k % 2 == 0:
                            nc.vector.tensor_copy(dst, ps_l[j])
                        else:
                            nc.scalar.activation(
                                out=dst, in_=ps_l[j],
                                func=mybir.ActivationFunctionType.Copy,
                            )
                        k += 1
            else:
                nc.sync.dma_start(
                    out=tt_g,
                    in_=tt[b, SHIP_IDX[g]].rearrange("p (c n) -> p c n", c=DC),
                )

            s_ps = psum_s.tile([P, 512], mybir.dt.float32)
            for c in range(DC):
                nc.tensor.matmul(
                    s_ps,
                    lhsT=qt_t[:, c, :],
                    rhs=tt_g[:, c, :],
                    start=(c == 0),
                    stop=(c == DC - 1),
                )

            # P = exp(S * scale), cast to bf16, chunked for ptr overlap.
            p_sb = p_pool.tile([P, 512], mybir.dt.bfloat16)
            for j in range(4):
                nc.scalar.activation(
                    out=p_sb[:, j * 128:(j + 1) * 128],
                    in_=s_ps[:, j * 128:(j + 1) * 128],
                    func=mybir.ActivationFunctionType.Exp,
                    scale=SCALE,
                )

            if len(tr_q) == 2:
                if mm2_q:
                    mm2_stage(*mm2_q.pop(0))
                p_sb0, tn_g0, b0, g0 = tr_q.pop(0)
                pt_sbs = transpose_stage(p_sb0, b0, g0)
                mm2_q.append((pt_sbs, tn_g0, b0, g0))
            tr_q.append((p_sb, tn_g, b, g))
        while tr_q:
            if mm2_q:
                mm2_stage(*mm2_q.pop(0))
            p_sb0, tn_g0, b0, g0 = tr_q.pop(0)
            pt_sbs = transpose_stage(p_sb0, b0, g0)
            mm2_q.append((pt_sbs, tn_g0, b0, g0))
        while mm2_q:
            mm2_stage(*mm2_q.pop(0))

    nc.compile()
    return nc


def _get_nc() -> bass.Bass:
    global _cached_nc
    if _cached_nc is None:
        _cached_nc = build_bass_program()
    return _cached_nc


def _make_in_maps(x_latent: np.ndarray, prompts_latent: np.ndarray):
    xb = x_latent.astype(BF16)                                    # [16, N, D]
    # tn: [b, g, p, t, d] = T[b, g*512 + t*128 + p, d], + ones column (d=768)
    tn5 = xb.reshape(B_TOTAL, G, 4, 128, D).transpose(0, 1, 3, 2, 4)
    tn_h = np.empty((B_TOTAL, G, 128, 4, D + 1), dtype=BF16)
    tn_h[..., :D] = tn5
    tn_h[..., D] = np.asarray(1.0, dtype=BF16)
    tn_h = np.ascontiguousarray(tn_h.reshape(B_TOTAL, G, 128, 4 * (D + 1)))

    # tt (shipped groups only): [b, i, p, c, n] = T[b, g_i*512 + n, c*128 + p]
    xT = np.ascontiguousarray(xb.transpose(0, 2, 1))              # [16, D, N]
    tt6 = xT.reshape(B_TOTAL, DC, 128, G, 512).transpose(0, 3, 2, 1, 4)
    tt_h = np.ascontiguousarray(
        tt6[:, SHIP_GROUPS].reshape(B_TOTAL, NSHIP, 128, DC * 512)
    )

    # qt: [b, p, c, m] = q[b, m, c*128 + p]
    qT = prompts_latent.astype(BF16).transpose(0, 2, 1)           # [16, D, P]
    qt_h = np.ascontiguousarray(
        qT.reshape(B_TOTAL, DC, 128, P).transpose(0, 2, 1, 3)
        .reshape(B_TOTAL, 128, DC * P)
    )

    return [
        {
            "qt": qt_h[c * BPC:(c + 1) * BPC],
            "tt": tt_h[c * BPC:(c + 1) * BPC],
            "tn": tn_h[c * BPC:(c + 1) * BPC],
        }
        for c in range(N_CORES)
    ]


def run(x_latent: np.ndarray, prompts_latent: np.ndarray, trace: bool = False):
    """Run on all 8 cores; returns (output [16, 64, 768] f32, BassKernelResults)."""
    nc = _get_nc()
    in_maps = _make_in_maps(np.asarray(x_latent), np.asarray(prompts_latent))
    res = run_bass_kernel_spmd(nc, in_maps, list(range(N_CORES)), trace=trace)
    out = np.concatenate([np.asarray(r["out"]) for r in res.results], axis=0)
    return out.astype(np.float32), res


def kernel(x_latent: np.ndarray, prompts_latent: np.ndarray) -> np.ndarray:
    out, _ = run(x_latent, prompts_latent, trace=False)
    return out
